# revision 10
# baseline (speedup 1.0000x reference)
"""HashEmbedder3D Trainium2 kernel.

Strategy: data-parallel over points (8 cores x 131072 points), tables
replicated per core. Per core, points live as [128 partitions, 1024 slots].

Levels are fetched two ways:
  - Coarse dense levels with res^3 <= 32768 (res 16/20/25/32): a host-built
    voxel-corner table V_l[voxel] = all 8 corners (16 f32, padded to 64) is
    fetched with ONE 256B dma_gather block per point per level (int16 block
    indices, wrapped + group-replicated per the SWDGE gather ucode layout).
    Interpolation is then fully vectorized over the tile.
  - Remaining dense levels (res^3 < T): 2 slab fetches per point of (res+2)
    rows via indirect DMA (one offset per partition per instruction -- the
    only offset layout the SWDGE ucode walks correctly).
  - Hash levels: 8 single-row fetches per point (row pair = 2 floats).

Interpolation on VectorE; indices/weights vectorized per tile. A tc.For_i
loop over slots keeps the program small for the per-slot gather levels.
"""
import math
import sys

import numpy as np

sys.path.insert(0, "/opt/trn_rl_repo")

from concourse import bacc, bass, mybir
import concourse.tile as tile
from concourse.library_config import mlp

# ---- problem constants (mirror of the reference formulas) ----
N_LEVELS = 16
F = 2
LOG2_T = 19
T = 1 << LOG2_T
BASE, FINEST = 16, 512
B_GROWTH = float(np.exp((np.log(np.float32(FINEST)) - np.log(np.float32(BASE))) / np.float32(N_LEVELS - 1)))
RES = [math.floor(BASE * B_GROWTH**i) for i in range(N_LEVELS)]
SIZES = [(r + 1) ** 3 if r**3 < T else T for r in RES]
OFFS = np.concatenate([[0], np.cumsum(SIZES)]).tolist()
TOTAL_ROWS = OFFS[-1]
PRIMES = [1, 2654435761, 805459861]
N_POINTS = 1048576
N_CORES = 8
P = 128

# levels served from the voxel-corner table (block idx must fit int16)
VOX_LV = [l for l in range(N_LEVELS) if RES[l] ** 3 <= 32768]
VOX_SIZES = [RES[l] ** 3 for l in VOX_LV]
VOX_OFFS = np.concatenate([[0], np.cumsum(VOX_SIZES)]).tolist()
VOX_TOTAL = VOX_OFFS[-1]
GATHER_CALL = 1024  # idx per dma_gather call (SWDGE scratch ring limit)

DT = mybir.dt
AL = mybir.AluOpType


def build_kernel(slots_total, tile_slots):
    """slots_total: points per partition; tile_slots: slots per outer tile."""
    n_outer = slots_total // tile_slots
    assert n_outer * tile_slots == slots_total
    K = tile_slots
    use_vox = (K * P) % GATHER_CALL == 0 and K % 16 == 0

    nc = bacc.Bacc(None, target_bir_lowering=False, debug=False)
    x_in = nc.dram_tensor("x", [P, slots_total, 3], DT.float32, kind="ExternalInput")
    tab = nc.dram_tensor("tables", [TOTAL_ROWS, F], DT.float32, kind="ExternalInput")
    vt = nc.dram_tensor("voxtabs", [VOX_TOTAL, 64], DT.float32, kind="ExternalInput")
    out = nc.dram_tensor("out", [P, slots_total, 32], DT.float32, kind="ExternalOutput")
    scr = {
        l: nc.dram_tensor(f"scr{l}", [P * tile_slots], DT.int16, kind="Internal")
        for l in (VOX_LV if use_vox else [])
    }

    vox_lv = VOX_LV if use_vox else []
    dense_lv = [l for l in range(N_LEVELS) if RES[l] ** 3 < T and l not in vox_lv]
    hash_lv = [l for l in range(N_LEVELS) if RES[l] ** 3 >= T]
    # idx_all columns: dense levels 2 each, hash levels 8 each
    idx_cols = {}
    col = 0
    for l in dense_lv:
        idx_cols[l] = col
        col += 2
    for l in hash_lv:
        idx_cols[l] = col
        col += 8
    NIDX = col

    with tile.TileContext(nc) as tc:
        with (
            tc.tile_pool(name="big", bufs=1) as bigp,
            tc.tile_pool(name="work", bufs=2) as workp,
            tc.tile_pool(name="gt", bufs=2) as gtp,
            tc.tile_pool(name="vox", bufs=2) as voxp,
            tc.tile_pool(name="voxe", bufs=1) as voxep,
        ):
            if use_vox:
                nc.gpsimd.load_library(mlp)
            x_sb = bigp.tile([P, slots_total, 3], DT.float32, tag="x_sb")
            nc.sync.dma_start(x_sb[:], x_in[:])

            idx_all = bigp.tile([P, tile_slots, NIDX], DT.int32, tag="idx_all")
            w_all = bigp.tile([P, tile_slots, N_LEVELS * 3], DT.float32, tag="w_all")
            out_sb = bigp.tile([P, tile_slots, 32], DT.float32, tag="out_sb")

            # fixed per-slot tiles used inside the loop (x2 for double-buffering
            # so slot u+1's gathers overlap slot u's interp)
            UN = 2
            idx_cur = [bigp.tile([P, NIDX], DT.int32, tag=f"idx_cur{u}", name=f"idx_cur{u}") for u in range(UN)]
            w_cur = [bigp.tile([P, N_LEVELS * 3], DT.float32, tag=f"w_cur{u}", name=f"w_cur{u}") for u in range(UN)]
            gd = {}
            for l in dense_lv:
                D = (RES[l] + 2) * 2
                gd[l] = [
                    (
                        bigp.tile([P, D], DT.float32, tag=f"gd{l}a{u}", name=f"gd{l}a{u}"),
                        bigp.tile([P, D], DT.float32, tag=f"gd{l}b{u}", name=f"gd{l}b{u}"),
                    )
                    for u in range(UN)
                ]
            gh = {
                l: [bigp.tile([P, 16], DT.float32, tag=f"gh{l}{u}", name=f"gh{l}{u}") for u in range(UN)]
                for l in hash_lv
            }
            # interp temps
            tmpA = {
                l: [bigp.tile([P, (RES[l] + 2) * 2], DT.float32, tag=f"tA{l}{u}", name=f"tA{l}{u}") for u in range(UN)]
                for l in dense_lv
            }
            tmpAh = [bigp.tile([P, 8], DT.float32, tag=f"tAh{u}", name=f"tAh{u}") for u in range(UN)]
            tmpB = [bigp.tile([P, 4], DT.float32, tag=f"tB{u}", name=f"tB{u}") for u in range(UN)]

            for t_out in range(n_outer):
                xs = x_sb[:, t_out * tile_slots : (t_out + 1) * tile_slots, :]

                # ---- vectorized index / weight computation ----
                xc = workp.tile([P, tile_slots, 3], DT.float32, tag="xc")
                nc.vector.tensor_scalar(out=xc[:], in0=xs, op0=AL.max, scalar1=-1.0, op1=AL.min, scalar2=1.0)
                tf = workp.tile([P, tile_slots, 3], DT.float32, tag="tf")
                fi = workp.tile([P, tile_slots, 3], DT.int32, tag="fi")
                ff = workp.tile([P, tile_slots, 3], DT.float32, tag="ff")
                blf = workp.tile([P, tile_slots, 3], DT.float32, tag="blf")
                bli = workp.tile([P, tile_slots, 3], DT.int32, tag="bli")
                su = workp.tile([P, tile_slots, 3], DT.float32, tag="su")

                for l in range(N_LEVELS):
                    res = RES[l]
                    grid = np.float32(2.0) / np.float32(res)
                    inv = np.float32(1.0) / grid
                    # t = (xc + 1) * inv
                    nc.vector.tensor_scalar(out=tf[:], in0=xc[:], op0=AL.add, scalar1=1.0, op1=AL.mult, scalar2=float(inv))
                    # floor via trunc + correction (trunc==floor for t>=0; guard rounding)
                    nc.vector.tensor_copy(out=fi[:], in_=tf[:])
                    nc.vector.tensor_copy(out=ff[:], in_=fi[:])
                    nc.vector.tensor_tensor(out=blf[:], in0=ff[:], in1=tf[:], op=AL.is_gt)
                    nc.vector.tensor_tensor(out=blf[:], in0=ff[:], in1=blf[:], op=AL.subtract)
                    # clamp to [0, res-1]
                    nc.vector.tensor_scalar(out=blf[:], in0=blf[:], op0=AL.max, scalar1=0.0, op1=AL.min, scalar2=float(res - 1))
                    nc.vector.tensor_copy(out=bli[:], in_=blf[:])
                    # w = (x - (blf*grid - 1)) * inv  (uses unclipped x)
                    nc.vector.tensor_scalar(out=su[:], in0=blf[:], op0=AL.mult, scalar1=float(grid), scalar2=None)
                    nc.vector.tensor_tensor(out=su[:], in0=xs, in1=su[:], op=AL.subtract)
                    nc.vector.tensor_scalar(
                        out=w_all[:, :, l * 3 : (l + 1) * 3], in0=su[:], op0=AL.add, scalar1=1.0, op1=AL.mult, scalar2=float(inv)
                    )

                    i_ = bli[:, :, 0:1]
                    j_ = bli[:, :, 1:2]
                    k_ = bli[:, :, 2:3]
                    if l in vox_lv:
                        # ---- voxel-corner-table path: one 256B block per point ----
                        lv = VOX_LV.index(l)
                        v32 = voxp.tile([P, K], DT.int32, tag="v32", name=f"v32_{t_out}_{l}")
                        v16 = voxp.tile([P, K], DT.int16, tag="v16", name=f"v16_{t_out}_{l}")
                        t1v = workp.tile([P, tile_slots, 1], DT.int32, tag="t1v")
                        t2v = workp.tile([P, tile_slots, 1], DT.int32, tag="t2v")
                        nc.vector.tensor_scalar(out=t1v[:], in0=j_, op0=AL.mult, scalar1=res, scalar2=None)
                        nc.vector.tensor_tensor(out=t1v[:], in0=t1v[:], in1=k_, op=AL.add)
                        nc.vector.tensor_scalar(out=t2v[:], in0=i_, op0=AL.mult, scalar1=res * res, scalar2=None)
                        nc.vector.tensor_tensor(
                            out=v32[:].unsqueeze(2), in0=t2v[:], in1=t1v[:], op=AL.add
                        )
                        nc.vector.tensor_copy(out=v16[:], in_=v32[:])
                        # wrap to SWDGE gather idx layout: value for gather slot
                        # n = m*128 + p goes to partition n%16, col m*8 + p//16,
                        # replicated to all 8 gpsimd core groups.
                        idxs = voxp.tile([P, K * 8], DT.int16, tag="idxs", name=f"idxs_{t_out}_{l}")
                        nc.sync.dma_start(scr[l][:], v16[:])
                        for j in range(8):
                            nc.sync.dma_start(
                                idxs[0:16, :].rearrange("q (m j) -> q m j", j=8)[:, :, j : j + 1],
                                scr[l][16 * j * K : (16 * j + 16) * K].rearrange("(q m) -> q m", q=16),
                            )
                        for g in range(1, 8):
                            nc.sync.dma_start(idxs[16 * g : 16 * (g + 1), :], idxs[0:16, :])
                        eV = voxep.tile([P, K, 64], DT.float32, tag="eV", name=f"eV_{t_out}_{l}")
                        SL = GATHER_CALL // P
                        for c in range((K * P) // GATHER_CALL):
                            nc.gpsimd.dma_gather(
                                eV[:, c * SL : (c + 1) * SL, :],
                                vt[VOX_OFFS[lv] : VOX_OFFS[lv + 1], :],
                                idxs[:, c * SL * 8 : (c + 1) * SL * 8],
                                GATHER_CALL,
                                GATHER_CALL,
                                64,
                            )
                        # vectorized trilinear interp over the tile
                        wxb = w_all[:, :, l * 3 + 0 : l * 3 + 1].broadcast_to([P, K, 8])
                        wyb = w_all[:, :, l * 3 + 1 : l * 3 + 2].broadcast_to([P, K, 4])
                        wzb = w_all[:, :, l * 3 + 2 : l * 3 + 3].broadcast_to([P, K, 2])
                        vA = voxp.tile([P, K, 8], DT.float32, tag="vA", name=f"vA_{t_out}_{l}")
                        vB = voxp.tile([P, K, 4], DT.float32, tag="vB", name=f"vB_{t_out}_{l}")
                        nc.vector.tensor_tensor(out=vA[:], in0=eV[:, :, 8:16], in1=eV[:, :, 0:8], op=AL.subtract)
                        nc.vector.tensor_tensor(out=vA[:], in0=vA[:], in1=wxb, op=AL.mult)
                        nc.vector.tensor_tensor(out=vA[:], in0=vA[:], in1=eV[:, :, 0:8], op=AL.add)
                        nc.vector.tensor_tensor(out=vB[:], in0=vA[:, :, 4:8], in1=vA[:, :, 0:4], op=AL.subtract)
                        nc.vector.tensor_tensor(out=vB[:], in0=vB[:], in1=wyb, op=AL.mult)
                        nc.vector.tensor_tensor(out=vB[:], in0=vB[:], in1=vA[:, :, 0:4], op=AL.add)
                        ovx = out_sb[:, :, l * 2 : l * 2 + 2]
                        nc.vector.tensor_tensor(out=ovx, in0=vB[:, :, 2:4], in1=vB[:, :, 0:2], op=AL.subtract)
                        nc.vector.tensor_tensor(out=ovx, in0=ovx, in1=wzb, op=AL.mult)
                        nc.vector.tensor_tensor(out=ovx, in0=ovx, in1=vB[:, :, 0:2], op=AL.add)
                        continue

                    c0 = idx_cols[l]
                    if res**3 < T:
                        # slab bases: b_di = (i+di)*res^2 + j*res + k
                        t1 = workp.tile([P, tile_slots, 1], DT.int32, tag="t1")
                        t2 = workp.tile([P, tile_slots, 1], DT.int32, tag="t2")
                        nc.vector.tensor_scalar(out=t1[:], in0=j_, op0=AL.mult, scalar1=res, scalar2=None)
                        nc.vector.tensor_tensor(out=t1[:], in0=t1[:], in1=k_, op=AL.add)
                        nc.vector.tensor_scalar(out=t2[:], in0=i_, op0=AL.mult, scalar1=res * res, scalar2=None)
                        nc.vector.tensor_tensor(
                            out=idx_all[:, :, c0 : c0 + 1], in0=t2[:], in1=t1[:], op=AL.add
                        )
                        nc.vector.tensor_scalar(
                            out=idx_all[:, :, c0 + 1 : c0 + 2], in0=idx_all[:, :, c0 : c0 + 1], op0=AL.add, scalar1=res * res
                        , scalar2=None)
                    else:
                        jp = workp.tile([P, tile_slots, 2], DT.int32, tag="jp")
                        kp = workp.tile([P, tile_slots, 2], DT.int32, tag="kp")
                        ii = workp.tile([P, tile_slots, 2], DT.int32, tag="ii")
                        mt1 = workp.tile([P, tile_slots, 1], DT.int32, tag="mt1")
                        mt2 = workp.tile([P, tile_slots, 1], DT.int32, tag="mt2")
                        mt3 = workp.tile([P, tile_slots, 1], DT.int32, tag="mt3")

                        def ts(out_, in_, op, s):
                            nc.vector.tensor_scalar(out=out_, in0=in_, op0=op, scalar1=s, scalar2=None)

                        def tt(out_, a, b, op):
                            nc.vector.tensor_tensor(out=out_, in0=a, in1=b, op=op)

                        def mul32(dst, src, prime):
                            # dst = (src * prime) mod 2^32, exact via 16-bit limbs.
                            # DVE int mult/add are fp32 (exact < 2^24); src <= 512.
                            Hp, Lp = (prime >> 16) & 0xFFFF, prime & 0xFFFF
                            Hs = Hp - 32768 if Hp >= 32768 else Hp
                            # mt1 = src*L (exact, <=2^24)
                            ts(mt1[:], src, AL.mult, Lp)
                            # mt2 = (src*H) & 0xFFFF via (src*H' + (src<<15 if Hp>=2^15)) mod 2^16
                            ts(mt2[:], src, AL.mult, Hs)
                            if Hp >= 32768:
                                ts(mt3[:], src, AL.logical_shift_left, 15)
                                ts(mt3[:], mt3[:], AL.bitwise_and, 0xFFFF)
                                ts(mt2[:], mt2[:], AL.bitwise_and, 0xFFFF)
                                tt(mt2[:], mt2[:], mt3[:], AL.add)
                            ts(mt2[:], mt2[:], AL.bitwise_and, 0xFFFF)
                            # hi16 = ((src*L)>>16 + mt2) & 0xFFFF
                            ts(mt3[:], mt1[:], AL.logical_shift_right, 16)
                            tt(mt2[:], mt2[:], mt3[:], AL.add)
                            ts(mt2[:], mt2[:], AL.bitwise_and, 0xFFFF)
                            # dst = (hi16<<16) | (lo16)
                            ts(mt2[:], mt2[:], AL.logical_shift_left, 16)
                            ts(mt1[:], mt1[:], AL.bitwise_and, 0xFFFF)
                            tt(dst, mt2[:], mt1[:], AL.bitwise_or)

                        def add32(dst, src, const):
                            # dst = (src + const) mod 2^32 exact via limbs
                            cl, ch = const & 0xFFFF, (const >> 16) & 0xFFFF
                            ts(mt1[:], src, AL.bitwise_and, 0xFFFF)
                            ts(mt1[:], mt1[:], AL.add, cl)  # <= 2^17 exact
                            ts(mt2[:], src, AL.logical_shift_right, 16)
                            ts(mt2[:], mt2[:], AL.bitwise_and, 0xFFFF)
                            ts(mt2[:], mt2[:], AL.add, ch)
                            ts(mt3[:], mt1[:], AL.logical_shift_right, 16)  # carry
                            tt(mt2[:], mt2[:], mt3[:], AL.add)
                            ts(mt2[:], mt2[:], AL.bitwise_and, 0xFFFF)
                            ts(mt2[:], mt2[:], AL.logical_shift_left, 16)
                            ts(mt1[:], mt1[:], AL.bitwise_and, 0xFFFF)
                            tt(dst, mt2[:], mt1[:], AL.bitwise_or)

                        mul32(jp[:, :, 0:1], j_, PRIMES[1])
                        add32(jp[:, :, 1:2], jp[:, :, 0:1], PRIMES[1])
                        mul32(kp[:, :, 0:1], k_, PRIMES[2])
                        add32(kp[:, :, 1:2], kp[:, :, 0:1], PRIMES[2])
                        nc.vector.tensor_copy(out=ii[:, :, 0:1], in_=i_)
                        nc.vector.tensor_scalar(out=ii[:, :, 1:2], in0=i_, op0=AL.add, scalar1=1, scalar2=None)
                        m = 0
                        for di in (0, 1):
                            for dj in (0, 1):
                                for dk in (0, 1):
                                    dst = idx_all[:, :, c0 + m : c0 + m + 1]
                                    nc.vector.tensor_tensor(
                                        out=dst, in0=ii[:, :, di : di + 1], in1=jp[:, :, dj : dj + 1], op=AL.bitwise_xor
                                    )
                                    nc.vector.tensor_tensor(out=dst, in0=dst, in1=kp[:, :, dk : dk + 1], op=AL.bitwise_xor)
                                    nc.vector.tensor_scalar(out=dst, in0=dst, op0=AL.bitwise_and, scalar1=T - 1, scalar2=None)
                                    m += 1

                # ---- per-slot gather + interp loop (2 slots/iter, dbl-buffered) ----
                assert tile_slots % 2 == 0
                with tc.For_i(0, tile_slots, 2, hint_engines=(mybir.EngineType.DVE, mybir.EngineType.Pool)) as c:
                    for u in range(2):
                        cs = c + u
                        nc.vector.tensor_copy(out=idx_cur[u][:], in_=idx_all[:, bass.ds(cs, 1), :])
                        nc.vector.tensor_copy(out=w_cur[u][:], in_=w_all[:, bass.ds(cs, 1), :])
                        for l in dense_lv:
                            c0 = idx_cols[l]
                            for di in (0, 1):
                                nc.gpsimd.indirect_dma_start(
                                    out=gd[l][u][di][:],
                                    out_offset=None,
                                    in_=tab[:],
                                    in_offset=bass.IndirectOffsetOnAxis(ap=idx_cur[u][:, c0 + di : c0 + di + 1], axis=0),
                                    element_offset=OFFS[l] * F,
                                )
                        for l in hash_lv:
                            c0 = idx_cols[l]
                            for m in range(8):
                                nc.gpsimd.indirect_dma_start(
                                    out=gh[l][u][:, m * 2 : (m + 1) * 2],
                                    out_offset=None,
                                    in_=tab[:],
                                    in_offset=bass.IndirectOffsetOnAxis(ap=idx_cur[u][:, c0 + m : c0 + m + 1], axis=0),
                                    element_offset=OFFS[l] * F,
                                )
                        # interp
                        for l in dense_lv + hash_lv:
                            res = RES[l]
                            wx = w_cur[u][:, l * 3 + 0 : l * 3 + 1]
                            wy = w_cur[u][:, l * 3 + 1 : l * 3 + 2]
                            wz = w_cur[u][:, l * 3 + 2 : l * 3 + 3]
                            if res**3 < T:
                                D = (res + 2) * 2
                                A = tmpA[l][u]
                                nc.vector.tensor_tensor(out=A[:], in0=gd[l][u][1][:], in1=gd[l][u][0][:], op=AL.subtract)
                                nc.vector.tensor_tensor(out=A[:], in0=A[:], in1=wx.to_broadcast([P, D]), op=AL.mult)
                                nc.vector.tensor_tensor(out=A[:], in0=A[:], in1=gd[l][u][0][:], op=AL.add)
                                B = tmpB[u]
                                for dk in (0, 1):
                                    a0 = A[:, dk * 2 : dk * 2 + 2]
                                    a1 = A[:, (res + dk) * 2 : (res + dk) * 2 + 2]
                                    bo = B[:, dk * 2 : (dk + 1) * 2]
                                    nc.vector.tensor_tensor(out=bo, in0=a1, in1=a0, op=AL.subtract)
                                    nc.vector.tensor_tensor(out=bo, in0=bo, in1=wy.to_broadcast([P, 2]), op=AL.mult)
                                    nc.vector.tensor_tensor(out=bo, in0=bo, in1=a0, op=AL.add)
                            else:
                                A8 = tmpAh[u]
                                g = gh[l][u]
                                nc.vector.tensor_tensor(out=A8[:], in0=g[:, 8:16], in1=g[:, 0:8], op=AL.subtract)
                                nc.vector.tensor_tensor(out=A8[:], in0=A8[:], in1=wx.to_broadcast([P, 8]), op=AL.mult)
                                nc.vector.tensor_tensor(out=A8[:], in0=A8[:], in1=g[:, 0:8], op=AL.add)
                                B = tmpB[u]
                                nc.vector.tensor_tensor(out=B[:], in0=A8[:, 4:8], in1=A8[:, 0:4], op=AL.subtract)
                                nc.vector.tensor_tensor(out=B[:], in0=B[:], in1=wy.to_broadcast([P, 4]), op=AL.mult)
                                nc.vector.tensor_tensor(out=B[:], in0=B[:], in1=A8[:, 0:4], op=AL.add)
                            oslot = out_sb[:, bass.ds(cs, 1), l * 2 : l * 2 + 2]
                            ztile = gtp.tile([P, 2], DT.float32, tag=f"zt{u}", name=f"zt{u}")
                            nc.vector.tensor_tensor(out=ztile[:], in0=B[:, 2:4], in1=B[:, 0:2], op=AL.subtract)
                            nc.vector.tensor_tensor(out=ztile[:], in0=ztile[:], in1=wz.to_broadcast([P, 2]), op=AL.mult)
                            nc.vector.tensor_tensor(out=oslot, in0=ztile[:], in1=B[:, 0:2], op=AL.add)

                nc.sync.dma_start(out[:, t_out * tile_slots : (t_out + 1) * tile_slots, :], out_sb[:])
    nc.compile()
    return nc


def build_voxtabs(tables: np.ndarray) -> np.ndarray:
    """V_l[voxel v=i*res^2+j*res+k, (di*4+dj*2+dk)*2+f] = level table at the
    corner rows, padded to 64 f32 (one 256B dma_gather block per voxel)."""
    parts = []
    for l in VOX_LV:
        res = RES[l]
        tabl = tables[OFFS[l] : OFFS[l + 1]]
        i, j, k = np.meshgrid(np.arange(res), np.arange(res), np.arange(res), indexing="ij")
        base = (i * res * res + j * res + k).ravel()
        V = np.zeros((res**3, 64), np.float32)
        col = 0
        for di in (0, 1):
            for dj in (0, 1):
                for dk in (0, 1):
                    rows = base + di * res * res + dj * res + dk
                    V[:, col : col + 2] = tabl[rows]
                    col += 2
        parts.append(V)
    return np.ascontiguousarray(np.concatenate(parts, axis=0))


_NC_CACHE = {}


def _get_nc(slots_total, tile_slots):
    key = (slots_total, tile_slots)
    if key not in _NC_CACHE:
        _NC_CACHE[key] = build_kernel(slots_total, tile_slots)
    return _NC_CACHE[key]


def kernel(x: np.ndarray, tables: np.ndarray) -> np.ndarray:
    from concourse.bass_utils import run_bass_kernel_spmd

    B = x.shape[0]
    per_core = B // N_CORES
    slots = per_core // P
    tile_slots = min(128, slots)
    nc = _get_nc(slots, tile_slots)
    tabf = np.ascontiguousarray(tables.astype(np.float32))
    voxtabs = build_voxtabs(tabf)
    in_maps = []
    for c in range(N_CORES):
        xs = np.ascontiguousarray(x[c * per_core : (c + 1) * per_core].reshape(P, slots, 3)).astype(np.float32)
        in_maps.append({"x": xs, "tables": tabf, "voxtabs": voxtabs})
    res = run_bass_kernel_spmd(nc, in_maps, core_ids=list(range(N_CORES)))
    outs = [res.results[c]["out"].reshape(per_core, 32) for c in range(N_CORES)]
    return np.concatenate(outs, axis=0).astype(np.float32)


# revision 11
# speedup vs baseline: 1.0176x; 1.0176x over previous
"""HashEmbedder3D Trainium2 kernel.

Strategy: data-parallel over points (8 cores x 131072 points), tables
replicated per core. Per core, points live as [128 partitions, 1024 slots].

Levels are fetched two ways:
  - Coarse dense levels with res^3 <= 32768 (res 16/20/25/32): a host-built
    voxel-corner table V_l[voxel] = all 8 corners (16 f32, padded to 64) is
    fetched with ONE 256B dma_gather block per point per level (int16 block
    indices, wrapped + group-replicated per the SWDGE gather ucode layout).
    Interpolation is then fully vectorized over the tile.
  - Remaining dense levels (res^3 < T): 2 slab fetches per point of (res+2)
    rows via indirect DMA (one offset per partition per instruction -- the
    only offset layout the SWDGE ucode walks correctly).
  - Hash levels: 8 single-row fetches per point (row pair = 2 floats).

Interpolation on VectorE; indices/weights vectorized per tile. A tc.For_i
loop over slots keeps the program small for the per-slot gather levels.
"""
import math
import sys

import numpy as np

sys.path.insert(0, "/opt/trn_rl_repo")

from concourse import bacc, bass, mybir
import concourse.tile as tile
from concourse.library_config import mlp

# ---- problem constants (mirror of the reference formulas) ----
N_LEVELS = 16
F = 2
LOG2_T = 19
T = 1 << LOG2_T
BASE, FINEST = 16, 512
B_GROWTH = float(np.exp((np.log(np.float32(FINEST)) - np.log(np.float32(BASE))) / np.float32(N_LEVELS - 1)))
RES = [math.floor(BASE * B_GROWTH**i) for i in range(N_LEVELS)]
SIZES = [(r + 1) ** 3 if r**3 < T else T for r in RES]
OFFS = np.concatenate([[0], np.cumsum(SIZES)]).tolist()
TOTAL_ROWS = OFFS[-1]
PRIMES = [1, 2654435761, 805459861]
N_POINTS = 1048576
N_CORES = 8
P = 128

# levels served from the voxel-corner table (block idx must fit int16)
VOX_LV = [l for l in range(N_LEVELS) if RES[l] ** 3 <= 32768]
VOX_SIZES = [RES[l] ** 3 for l in VOX_LV]
VOX_OFFS = np.concatenate([[0], np.cumsum(VOX_SIZES)]).tolist()
VOX_TOTAL = VOX_OFFS[-1]
GATHER_CALL = 1024  # idx per dma_gather call (SWDGE scratch ring limit)

DT = mybir.dt
AL = mybir.AluOpType


def build_kernel(slots_total, tile_slots):
    """slots_total: points per partition; tile_slots: slots per outer tile."""
    n_outer = slots_total // tile_slots
    assert n_outer * tile_slots == slots_total
    K = tile_slots
    use_vox = (K * P) % GATHER_CALL == 0 and K % 16 == 0

    nc = bacc.Bacc(None, target_bir_lowering=False, debug=False)
    x_in = nc.dram_tensor("x", [P, slots_total, 3], DT.float32, kind="ExternalInput")
    tab = nc.dram_tensor("tables", [TOTAL_ROWS, F], DT.float32, kind="ExternalInput")
    vt = nc.dram_tensor("voxtabs", [VOX_TOTAL, 64], DT.float32, kind="ExternalInput")
    out = nc.dram_tensor("out", [P, slots_total, 32], DT.float32, kind="ExternalOutput")
    scr = {
        l: nc.dram_tensor(f"scr{l}", [P * tile_slots], DT.int16, kind="Internal")
        for l in (VOX_LV if use_vox else [])
    }

    vox_lv = VOX_LV if use_vox else []
    dense_lv = [l for l in range(N_LEVELS) if RES[l] ** 3 < T and l not in vox_lv]
    hash_lv = [l for l in range(N_LEVELS) if RES[l] ** 3 >= T]
    # idx_all columns: dense levels 2 each, hash levels 8 each
    idx_cols = {}
    col = 0
    for l in dense_lv:
        idx_cols[l] = col
        col += 2
    for l in hash_lv:
        idx_cols[l] = col
        col += 8
    NIDX = col

    with tile.TileContext(nc) as tc:
        with (
            tc.tile_pool(name="big", bufs=1) as bigp,
            tc.tile_pool(name="work", bufs=2) as workp,
            tc.tile_pool(name="gt", bufs=2) as gtp,
            tc.tile_pool(name="vox", bufs=2) as voxp,
            tc.tile_pool(name="voxe", bufs=1) as voxep,
        ):
            if use_vox:
                nc.gpsimd.load_library(mlp)
            x_sb = bigp.tile([P, slots_total, 3], DT.float32, tag="x_sb")
            nc.sync.dma_start(x_sb[:], x_in[:])

            idx_all = bigp.tile([P, tile_slots, NIDX], DT.int32, tag="idx_all")
            w_all = bigp.tile([P, tile_slots, N_LEVELS * 3], DT.float32, tag="w_all")
            out_sb = bigp.tile([P, tile_slots, 32], DT.float32, tag="out_sb")

            # fixed per-slot tiles used inside the loop (x2 for double-buffering
            # so slot u+1's gathers overlap slot u's interp)
            UN = 4
            idx_cur = [bigp.tile([P, NIDX], DT.int32, tag=f"idx_cur{u}", name=f"idx_cur{u}") for u in range(UN)]
            w_cur = [bigp.tile([P, N_LEVELS * 3], DT.float32, tag=f"w_cur{u}", name=f"w_cur{u}") for u in range(UN)]
            gd = {}
            for l in dense_lv:
                D = (RES[l] + 2) * 2
                gd[l] = [
                    (
                        bigp.tile([P, D], DT.float32, tag=f"gd{l}a{u}", name=f"gd{l}a{u}"),
                        bigp.tile([P, D], DT.float32, tag=f"gd{l}b{u}", name=f"gd{l}b{u}"),
                    )
                    for u in range(UN)
                ]
            gh = {
                l: [bigp.tile([P, 16], DT.float32, tag=f"gh{l}{u}", name=f"gh{l}{u}") for u in range(UN)]
                for l in hash_lv
            }
            # interp temps
            tmpA = {
                l: [bigp.tile([P, (RES[l] + 2) * 2], DT.float32, tag=f"tA{l}{u}", name=f"tA{l}{u}") for u in range(UN)]
                for l in dense_lv
            }
            tmpAh = [bigp.tile([P, 8], DT.float32, tag=f"tAh{u}", name=f"tAh{u}") for u in range(UN)]
            tmpB = [bigp.tile([P, 4], DT.float32, tag=f"tB{u}", name=f"tB{u}") for u in range(UN)]

            for t_out in range(n_outer):
                xs = x_sb[:, t_out * tile_slots : (t_out + 1) * tile_slots, :]

                # ---- vectorized index / weight computation ----
                xc = workp.tile([P, tile_slots, 3], DT.float32, tag="xc")
                nc.vector.tensor_scalar(out=xc[:], in0=xs, op0=AL.max, scalar1=-1.0, op1=AL.min, scalar2=1.0)
                tf = workp.tile([P, tile_slots, 3], DT.float32, tag="tf")
                fi = workp.tile([P, tile_slots, 3], DT.int32, tag="fi")
                ff = workp.tile([P, tile_slots, 3], DT.float32, tag="ff")
                blf = workp.tile([P, tile_slots, 3], DT.float32, tag="blf")
                bli = workp.tile([P, tile_slots, 3], DT.int32, tag="bli")
                su = workp.tile([P, tile_slots, 3], DT.float32, tag="su")

                for l in range(N_LEVELS):
                    res = RES[l]
                    grid = np.float32(2.0) / np.float32(res)
                    inv = np.float32(1.0) / grid
                    # t = (xc + 1) * inv
                    nc.vector.tensor_scalar(out=tf[:], in0=xc[:], op0=AL.add, scalar1=1.0, op1=AL.mult, scalar2=float(inv))
                    # floor via trunc + correction (trunc==floor for t>=0; guard rounding)
                    nc.vector.tensor_copy(out=fi[:], in_=tf[:])
                    nc.vector.tensor_copy(out=ff[:], in_=fi[:])
                    nc.vector.tensor_tensor(out=blf[:], in0=ff[:], in1=tf[:], op=AL.is_gt)
                    nc.vector.tensor_tensor(out=blf[:], in0=ff[:], in1=blf[:], op=AL.subtract)
                    # clamp to [0, res-1]
                    nc.vector.tensor_scalar(out=blf[:], in0=blf[:], op0=AL.max, scalar1=0.0, op1=AL.min, scalar2=float(res - 1))
                    nc.vector.tensor_copy(out=bli[:], in_=blf[:])
                    # w = (x - (blf*grid - 1)) * inv  (uses unclipped x)
                    nc.vector.tensor_scalar(out=su[:], in0=blf[:], op0=AL.mult, scalar1=float(grid), scalar2=None)
                    nc.vector.tensor_tensor(out=su[:], in0=xs, in1=su[:], op=AL.subtract)
                    nc.vector.tensor_scalar(
                        out=w_all[:, :, l * 3 : (l + 1) * 3], in0=su[:], op0=AL.add, scalar1=1.0, op1=AL.mult, scalar2=float(inv)
                    )

                    i_ = bli[:, :, 0:1]
                    j_ = bli[:, :, 1:2]
                    k_ = bli[:, :, 2:3]
                    if l in vox_lv:
                        # ---- voxel-corner-table path: one 256B block per point ----
                        lv = VOX_LV.index(l)
                        v32 = voxp.tile([P, K], DT.int32, tag="v32", name=f"v32_{t_out}_{l}")
                        v16 = voxp.tile([P, K], DT.int16, tag="v16", name=f"v16_{t_out}_{l}")
                        t1v = workp.tile([P, tile_slots, 1], DT.int32, tag="t1v")
                        t2v = workp.tile([P, tile_slots, 1], DT.int32, tag="t2v")
                        nc.vector.tensor_scalar(out=t1v[:], in0=j_, op0=AL.mult, scalar1=res, scalar2=None)
                        nc.vector.tensor_tensor(out=t1v[:], in0=t1v[:], in1=k_, op=AL.add)
                        nc.vector.tensor_scalar(out=t2v[:], in0=i_, op0=AL.mult, scalar1=res * res, scalar2=None)
                        nc.vector.tensor_tensor(
                            out=v32[:].unsqueeze(2), in0=t2v[:], in1=t1v[:], op=AL.add
                        )
                        nc.vector.tensor_copy(out=v16[:], in_=v32[:])
                        # wrap to SWDGE gather idx layout: value for gather slot
                        # n = m*128 + p goes to partition n%16, col m*8 + p//16,
                        # replicated to all 8 gpsimd core groups.
                        idxs = voxp.tile([P, K * 8], DT.int16, tag="idxs", name=f"idxs_{t_out}_{l}")
                        nc.sync.dma_start(scr[l][:], v16[:])
                        for j in range(8):
                            nc.sync.dma_start(
                                idxs[0:16, :].rearrange("q (m j) -> q m j", j=8)[:, :, j : j + 1],
                                scr[l][16 * j * K : (16 * j + 16) * K].rearrange("(q m) -> q m", q=16),
                            )
                        for g in range(1, 8):
                            nc.sync.dma_start(idxs[16 * g : 16 * (g + 1), :], idxs[0:16, :])
                        eV = voxep.tile([P, K, 64], DT.float32, tag="eV", name=f"eV_{t_out}_{l}")
                        SL = GATHER_CALL // P
                        for c in range((K * P) // GATHER_CALL):
                            nc.gpsimd.dma_gather(
                                eV[:, c * SL : (c + 1) * SL, :],
                                vt[VOX_OFFS[lv] : VOX_OFFS[lv + 1], :],
                                idxs[:, c * SL * 8 : (c + 1) * SL * 8],
                                GATHER_CALL,
                                GATHER_CALL,
                                64,
                            )
                        # vectorized trilinear interp over the tile
                        wxb = w_all[:, :, l * 3 + 0 : l * 3 + 1].broadcast_to([P, K, 8])
                        wyb = w_all[:, :, l * 3 + 1 : l * 3 + 2].broadcast_to([P, K, 4])
                        wzb = w_all[:, :, l * 3 + 2 : l * 3 + 3].broadcast_to([P, K, 2])
                        vA = voxp.tile([P, K, 8], DT.float32, tag="vA", name=f"vA_{t_out}_{l}")
                        vB = voxp.tile([P, K, 4], DT.float32, tag="vB", name=f"vB_{t_out}_{l}")
                        nc.vector.tensor_tensor(out=vA[:], in0=eV[:, :, 8:16], in1=eV[:, :, 0:8], op=AL.subtract)
                        nc.vector.tensor_tensor(out=vA[:], in0=vA[:], in1=wxb, op=AL.mult)
                        nc.vector.tensor_tensor(out=vA[:], in0=vA[:], in1=eV[:, :, 0:8], op=AL.add)
                        nc.vector.tensor_tensor(out=vB[:], in0=vA[:, :, 4:8], in1=vA[:, :, 0:4], op=AL.subtract)
                        nc.vector.tensor_tensor(out=vB[:], in0=vB[:], in1=wyb, op=AL.mult)
                        nc.vector.tensor_tensor(out=vB[:], in0=vB[:], in1=vA[:, :, 0:4], op=AL.add)
                        ovx = out_sb[:, :, l * 2 : l * 2 + 2]
                        nc.vector.tensor_tensor(out=ovx, in0=vB[:, :, 2:4], in1=vB[:, :, 0:2], op=AL.subtract)
                        nc.vector.tensor_tensor(out=ovx, in0=ovx, in1=wzb, op=AL.mult)
                        nc.vector.tensor_tensor(out=ovx, in0=ovx, in1=vB[:, :, 0:2], op=AL.add)
                        continue

                    c0 = idx_cols[l]
                    if res**3 < T:
                        # slab bases: b_di = (i+di)*res^2 + j*res + k
                        t1 = workp.tile([P, tile_slots, 1], DT.int32, tag="t1")
                        t2 = workp.tile([P, tile_slots, 1], DT.int32, tag="t2")
                        nc.vector.tensor_scalar(out=t1[:], in0=j_, op0=AL.mult, scalar1=res, scalar2=None)
                        nc.vector.tensor_tensor(out=t1[:], in0=t1[:], in1=k_, op=AL.add)
                        nc.vector.tensor_scalar(out=t2[:], in0=i_, op0=AL.mult, scalar1=res * res, scalar2=None)
                        nc.vector.tensor_tensor(
                            out=idx_all[:, :, c0 : c0 + 1], in0=t2[:], in1=t1[:], op=AL.add
                        )
                        nc.vector.tensor_scalar(
                            out=idx_all[:, :, c0 + 1 : c0 + 2], in0=idx_all[:, :, c0 : c0 + 1], op0=AL.add, scalar1=res * res
                        , scalar2=None)
                    else:
                        jp = workp.tile([P, tile_slots, 2], DT.int32, tag="jp")
                        kp = workp.tile([P, tile_slots, 2], DT.int32, tag="kp")
                        ii = workp.tile([P, tile_slots, 2], DT.int32, tag="ii")
                        mt1 = workp.tile([P, tile_slots, 1], DT.int32, tag="mt1")
                        mt2 = workp.tile([P, tile_slots, 1], DT.int32, tag="mt2")
                        mt3 = workp.tile([P, tile_slots, 1], DT.int32, tag="mt3")

                        def ts(out_, in_, op, s):
                            nc.vector.tensor_scalar(out=out_, in0=in_, op0=op, scalar1=s, scalar2=None)

                        def tt(out_, a, b, op):
                            nc.vector.tensor_tensor(out=out_, in0=a, in1=b, op=op)

                        def mul32(dst, src, prime):
                            # dst = (src * prime) mod 2^32, exact via 16-bit limbs.
                            # DVE int mult/add are fp32 (exact < 2^24); src <= 512.
                            Hp, Lp = (prime >> 16) & 0xFFFF, prime & 0xFFFF
                            Hs = Hp - 32768 if Hp >= 32768 else Hp
                            # mt1 = src*L (exact, <=2^24)
                            ts(mt1[:], src, AL.mult, Lp)
                            # mt2 = (src*H) & 0xFFFF via (src*H' + (src<<15 if Hp>=2^15)) mod 2^16
                            ts(mt2[:], src, AL.mult, Hs)
                            if Hp >= 32768:
                                ts(mt3[:], src, AL.logical_shift_left, 15)
                                ts(mt3[:], mt3[:], AL.bitwise_and, 0xFFFF)
                                ts(mt2[:], mt2[:], AL.bitwise_and, 0xFFFF)
                                tt(mt2[:], mt2[:], mt3[:], AL.add)
                            ts(mt2[:], mt2[:], AL.bitwise_and, 0xFFFF)
                            # hi16 = ((src*L)>>16 + mt2) & 0xFFFF
                            ts(mt3[:], mt1[:], AL.logical_shift_right, 16)
                            tt(mt2[:], mt2[:], mt3[:], AL.add)
                            ts(mt2[:], mt2[:], AL.bitwise_and, 0xFFFF)
                            # dst = (hi16<<16) | (lo16)
                            ts(mt2[:], mt2[:], AL.logical_shift_left, 16)
                            ts(mt1[:], mt1[:], AL.bitwise_and, 0xFFFF)
                            tt(dst, mt2[:], mt1[:], AL.bitwise_or)

                        def add32(dst, src, const):
                            # dst = (src + const) mod 2^32 exact via limbs
                            cl, ch = const & 0xFFFF, (const >> 16) & 0xFFFF
                            ts(mt1[:], src, AL.bitwise_and, 0xFFFF)
                            ts(mt1[:], mt1[:], AL.add, cl)  # <= 2^17 exact
                            ts(mt2[:], src, AL.logical_shift_right, 16)
                            ts(mt2[:], mt2[:], AL.bitwise_and, 0xFFFF)
                            ts(mt2[:], mt2[:], AL.add, ch)
                            ts(mt3[:], mt1[:], AL.logical_shift_right, 16)  # carry
                            tt(mt2[:], mt2[:], mt3[:], AL.add)
                            ts(mt2[:], mt2[:], AL.bitwise_and, 0xFFFF)
                            ts(mt2[:], mt2[:], AL.logical_shift_left, 16)
                            ts(mt1[:], mt1[:], AL.bitwise_and, 0xFFFF)
                            tt(dst, mt2[:], mt1[:], AL.bitwise_or)

                        mul32(jp[:, :, 0:1], j_, PRIMES[1])
                        add32(jp[:, :, 1:2], jp[:, :, 0:1], PRIMES[1])
                        mul32(kp[:, :, 0:1], k_, PRIMES[2])
                        add32(kp[:, :, 1:2], kp[:, :, 0:1], PRIMES[2])
                        nc.vector.tensor_copy(out=ii[:, :, 0:1], in_=i_)
                        nc.vector.tensor_scalar(out=ii[:, :, 1:2], in0=i_, op0=AL.add, scalar1=1, scalar2=None)
                        m = 0
                        for di in (0, 1):
                            for dj in (0, 1):
                                for dk in (0, 1):
                                    dst = idx_all[:, :, c0 + m : c0 + m + 1]
                                    nc.vector.tensor_tensor(
                                        out=dst, in0=ii[:, :, di : di + 1], in1=jp[:, :, dj : dj + 1], op=AL.bitwise_xor
                                    )
                                    nc.vector.tensor_tensor(out=dst, in0=dst, in1=kp[:, :, dk : dk + 1], op=AL.bitwise_xor)
                                    nc.vector.tensor_scalar(out=dst, in0=dst, op0=AL.bitwise_and, scalar1=T - 1, scalar2=None)
                                    m += 1

                # ---- per-slot gather + interp loop (2 slots/iter, dbl-buffered) ----
                assert tile_slots % 4 == 0
                with tc.For_i(0, tile_slots, 4, hint_engines=(mybir.EngineType.DVE, mybir.EngineType.Pool)) as c:
                    for u in range(4):
                        cs = c + u
                        nc.vector.tensor_copy(out=idx_cur[u][:], in_=idx_all[:, bass.ds(cs, 1), :])
                        nc.vector.tensor_copy(out=w_cur[u][:], in_=w_all[:, bass.ds(cs, 1), :])
                        for l in dense_lv:
                            c0 = idx_cols[l]
                            for di in (0, 1):
                                nc.gpsimd.indirect_dma_start(
                                    out=gd[l][u][di][:],
                                    out_offset=None,
                                    in_=tab[:],
                                    in_offset=bass.IndirectOffsetOnAxis(ap=idx_cur[u][:, c0 + di : c0 + di + 1], axis=0),
                                    element_offset=OFFS[l] * F,
                                )
                        for l in hash_lv:
                            c0 = idx_cols[l]
                            for m in range(8):
                                nc.gpsimd.indirect_dma_start(
                                    out=gh[l][u][:, m * 2 : (m + 1) * 2],
                                    out_offset=None,
                                    in_=tab[:],
                                    in_offset=bass.IndirectOffsetOnAxis(ap=idx_cur[u][:, c0 + m : c0 + m + 1], axis=0),
                                    element_offset=OFFS[l] * F,
                                )
                        # interp
                        for l in dense_lv + hash_lv:
                            res = RES[l]
                            wx = w_cur[u][:, l * 3 + 0 : l * 3 + 1]
                            wy = w_cur[u][:, l * 3 + 1 : l * 3 + 2]
                            wz = w_cur[u][:, l * 3 + 2 : l * 3 + 3]
                            if res**3 < T:
                                D = (res + 2) * 2
                                A = tmpA[l][u]
                                nc.vector.tensor_tensor(out=A[:], in0=gd[l][u][1][:], in1=gd[l][u][0][:], op=AL.subtract)
                                nc.vector.tensor_tensor(out=A[:], in0=A[:], in1=wx.to_broadcast([P, D]), op=AL.mult)
                                nc.vector.tensor_tensor(out=A[:], in0=A[:], in1=gd[l][u][0][:], op=AL.add)
                                B = tmpB[u]
                                for dk in (0, 1):
                                    a0 = A[:, dk * 2 : dk * 2 + 2]
                                    a1 = A[:, (res + dk) * 2 : (res + dk) * 2 + 2]
                                    bo = B[:, dk * 2 : (dk + 1) * 2]
                                    nc.vector.tensor_tensor(out=bo, in0=a1, in1=a0, op=AL.subtract)
                                    nc.vector.tensor_tensor(out=bo, in0=bo, in1=wy.to_broadcast([P, 2]), op=AL.mult)
                                    nc.vector.tensor_tensor(out=bo, in0=bo, in1=a0, op=AL.add)
                            else:
                                A8 = tmpAh[u]
                                g = gh[l][u]
                                nc.vector.tensor_tensor(out=A8[:], in0=g[:, 8:16], in1=g[:, 0:8], op=AL.subtract)
                                nc.vector.tensor_tensor(out=A8[:], in0=A8[:], in1=wx.to_broadcast([P, 8]), op=AL.mult)
                                nc.vector.tensor_tensor(out=A8[:], in0=A8[:], in1=g[:, 0:8], op=AL.add)
                                B = tmpB[u]
                                nc.vector.tensor_tensor(out=B[:], in0=A8[:, 4:8], in1=A8[:, 0:4], op=AL.subtract)
                                nc.vector.tensor_tensor(out=B[:], in0=B[:], in1=wy.to_broadcast([P, 4]), op=AL.mult)
                                nc.vector.tensor_tensor(out=B[:], in0=B[:], in1=A8[:, 0:4], op=AL.add)
                            oslot = out_sb[:, bass.ds(cs, 1), l * 2 : l * 2 + 2]
                            ztile = gtp.tile([P, 2], DT.float32, tag=f"zt{u}", name=f"zt{u}")
                            nc.vector.tensor_tensor(out=ztile[:], in0=B[:, 2:4], in1=B[:, 0:2], op=AL.subtract)
                            nc.vector.tensor_tensor(out=ztile[:], in0=ztile[:], in1=wz.to_broadcast([P, 2]), op=AL.mult)
                            nc.vector.tensor_tensor(out=oslot, in0=ztile[:], in1=B[:, 0:2], op=AL.add)

                nc.sync.dma_start(out[:, t_out * tile_slots : (t_out + 1) * tile_slots, :], out_sb[:])
    nc.compile()
    return nc


def build_voxtabs(tables: np.ndarray) -> np.ndarray:
    """V_l[voxel v=i*res^2+j*res+k, (di*4+dj*2+dk)*2+f] = level table at the
    corner rows, padded to 64 f32 (one 256B dma_gather block per voxel)."""
    parts = []
    for l in VOX_LV:
        res = RES[l]
        tabl = tables[OFFS[l] : OFFS[l + 1]]
        i, j, k = np.meshgrid(np.arange(res), np.arange(res), np.arange(res), indexing="ij")
        base = (i * res * res + j * res + k).ravel()
        V = np.zeros((res**3, 64), np.float32)
        col = 0
        for di in (0, 1):
            for dj in (0, 1):
                for dk in (0, 1):
                    rows = base + di * res * res + dj * res + dk
                    V[:, col : col + 2] = tabl[rows]
                    col += 2
        parts.append(V)
    return np.ascontiguousarray(np.concatenate(parts, axis=0))


_NC_CACHE = {}


def _get_nc(slots_total, tile_slots):
    key = (slots_total, tile_slots)
    if key not in _NC_CACHE:
        _NC_CACHE[key] = build_kernel(slots_total, tile_slots)
    return _NC_CACHE[key]


def kernel(x: np.ndarray, tables: np.ndarray) -> np.ndarray:
    from concourse.bass_utils import run_bass_kernel_spmd

    B = x.shape[0]
    per_core = B // N_CORES
    slots = per_core // P
    tile_slots = min(128, slots)
    nc = _get_nc(slots, tile_slots)
    tabf = np.ascontiguousarray(tables.astype(np.float32))
    voxtabs = build_voxtabs(tabf)
    in_maps = []
    for c in range(N_CORES):
        xs = np.ascontiguousarray(x[c * per_core : (c + 1) * per_core].reshape(P, slots, 3)).astype(np.float32)
        in_maps.append({"x": xs, "tables": tabf, "voxtabs": voxtabs})
    res = run_bass_kernel_spmd(nc, in_maps, core_ids=list(range(N_CORES)))
    outs = [res.results[c]["out"].reshape(per_core, 32) for c in range(N_CORES)]
    return np.concatenate(outs, axis=0).astype(np.float32)
